# revision 38
# baseline (speedup 1.0000x reference)
"""GQA attention kernel for Trainium2 (8 NeuronCores, Bass/Tile).

Problem: B=2, S=2048, D=3072, 24 Q heads / 8 KV heads, HD=128, RoPE,
additive causal mask, softmax, output projection.

Sharding: tensor-parallel over heads. Core h owns KV head h and Q heads
{3h, 3h+1, 3h+2} for BOTH batch elements. Each core produces a partial
y^T = wo_slice^T.T @ attn_out_heads^T of shape (B, D, S) in fp16; the
host sums the 8 partials in fp32 and transposes back.

Layout: everything transposed ([feature, token]) on chip so every
matmul contracts on the partition dim with a 512-wide fp16 moving
operand (1 cycle/row on the PE):
  - x^T streamed from DRAM (host pre-transposed, fp16)
  - QKV projection -> Q^T,K^T [hd, S] per head; RoPE in transposed
    layout (rotate-half via partition-shifted SBUF DMA, sign folded
    into sinT)
  - scores^T [k, q] = K-tile @ Q^T into PAIRED PSUM banks; ONE exp per
    pair on ACT ([128,2,512]) with the 1/sqrt(HD) scale folded in;
    causal mask applied multiplicatively (fp16, DVE 4x mode)
  - attn@V accumulated in PSUM; softmax denominator built OFF the PE:
    DVE accumulates sum of exp tiles (fp16), GpSimd partition_all_reduce
    produces the row-sum broadcast to all partitions, DVE reciprocal +
    multiply normalize
  - out-projection matmuls interleaved into the NEXT q-chunk's
    score/AV loop so the ACT-bound k-loop and PE-bound out-proj overlap
"""

import math
import os
import sys

import numpy as np

for _p in ("/opt/trn_rl_repo",):
    if os.path.isdir(_p) and _p not in sys.path:
        sys.path.insert(0, _p)

import concourse.bass as bass  # noqa: E402
import concourse.bass_isa as bass_isa  # noqa: E402
import concourse.mybir as mybir  # noqa: E402
import concourse.tile as tile  # noqa: E402
from concourse import bacc  # noqa: E402
from concourse.bass_utils import run_bass_kernel_spmd  # noqa: E402

F32 = mybir.dt.float32
F16 = mybir.dt.float16
AFT = mybir.ActivationFunctionType

N_CORES = 8

# Set by test harness to capture a profile on the next kernel() call.
TRACE = False
LAST_EXEC_NS = None
LAST_RESULTS = None

B, S, D = 2, 2048, 3072
QH, HD, SC = 3, 128, 512
CT = D // 128          # 24 contraction tiles
KT = S // 128          # 16 key tiles
NSC = S // SC          # 4 token chunks
XG = 6                 # x tiles per DMA group
SCALE = 1.0 / math.sqrt(HD)


def build_program():
    nc = bacc.Bacc("TRN2", target_bir_lowering=False, debug=False,
                   num_devices=N_CORES)

    # All dram layouts host-repacked so every DMA has long contiguous
    # per-partition lines (fat packets) and needs no on-the-fly rearrange.
    xT = nc.declare_dram_parameter("xT", [B, NSC, 128, CT, SC], F16,
                                   isOutput=False)
    cosT = nc.declare_dram_parameter("cosT", [HD, S], F16, isOutput=False)
    sinT = nc.declare_dram_parameter("sinT", [HD, S], F16, isOutput=False)
    onesc = nc.declare_dram_parameter("onesc", [128, 1], F16, isOutput=False)
    onesr = nc.declare_dram_parameter("onesr", [1, 128], F16, isOutput=False)
    wq = nc.declare_dram_parameter("wq", [128, CT, QH * HD], F16,
                                   isOutput=False)
    wk = nc.declare_dram_parameter("wk", [128, CT, HD], F16, isOutput=False)
    wv = nc.declare_dram_parameter("wv", [128, CT, HD], F16, isOutput=False)
    wo = nc.declare_dram_parameter("wo", [128, QH, D], F16, isOutput=False)
    # 8 masked (qc, kt-pair) blocks of exp(mask)^T, fp16 {0,1}
    em2 = nc.declare_dram_parameter("em2", [128, 2 * NSC, 2, SC], F16,
                                    isOutput=False)
    ident = nc.declare_dram_parameter("ident", [128, 128], F16, isOutput=False)
    yT = nc.declare_dram_parameter("yT", [B, D, S], F16, isOutput=True)

    xT_ap, yT_ap = xT.ap(), yT.ap()

    with tile.TileContext(nc) as tc:
        from contextlib import ExitStack
        with ExitStack() as top:
            const = top.enter_context(tc.tile_pool(name="const", bufs=1))
            stream = top.enter_context(tc.tile_pool(name="stream", bufs=1))

            wq_sb = const.tile([128, CT, QH * HD], F16, name="wq_sb")
            wk_sb = const.tile([128, CT, HD], F16, name="wk_sb")
            wv_sb = const.tile([128, CT, HD], F16, name="wv_sb")
            wo_sb = const.tile([128, QH, D], F16, name="wo_sb")
            cos_sb = const.tile([128, S], F16, name="cos_sb")
            sin_sb = const.tile([128, S], F16, name="sin_sb")
            em_sb = const.tile([128, 2 * NSC, 2, SC], F16, name="em_sb")
            ident_sb = const.tile([128, 128], F16, name="ident_sb")
            ones_col = const.tile([128, 1], F16, name="ones_col")
            ones_row = const.tile([1, 128], F16, name="ones_row")

            # x chunk group tiles, cached so the next batch's first chunk
            # can be prefetched during the previous batch's attention.
            xg_cache = {}

            def prefetch_xgroups(b, sc, with_weights=False):
                key = (b, sc)
                if key not in xg_cache:
                    # Cold start uses fine 3-ct groups so the first matmuls
                    # gate on ~0.8MB of DMA instead of several MB.
                    plan = [3] * 8 if with_weights else [XG] * (CT // XG)
                    xgs, ct0 = [], 0
                    for n in plan:
                        gs = slice(ct0, ct0 + n)
                        if with_weights:
                            # rate-match weight and x arrival per group
                            nc.sync.dma_start(wq_sb[:, gs, :], wq.ap()[:, gs])
                            nc.sync.dma_start(wk_sb[:, gs, :], wk.ap()[:, gs])
                            nc.sync.dma_start(wv_sb[:, gs, :], wv.ap()[:, gs])
                        xg = stream.tile([128, n, SC], F16, tag="xg",
                                         bufs=4, name="xg",
                                         padded_shape=[128, XG, SC])
                        nc.sync.dma_start(xg[:], xT_ap[b, sc, :, gs, :])
                        xgs.append((xg, ct0, n))
                        ct0 += n
                    xg_cache[key] = xgs

            def get_xgroups(b, sc):
                prefetch_xgroups(b, sc)
                groups = xg_cache.pop((b, sc))
                xr_map = {}
                for xg, ct0, n in groups:
                    for k in range(n):
                        xr_map[ct0 + k] = xg[:, k, :]
                return xr_map

            prefetch_xgroups(0, 0, with_weights=True)

            def late_preloads(stage):
                # Staged behind the x chunks that precede their first use so
                # they never delay the proj matmul stream.
                if stage == 0:      # RoPE needs these from chunk 0
                    nc.sync.dma_start(cos_sb[:], cosT.ap())
                    nc.sync.dma_start(sin_sb[:], sinT.ap())
                    nc.sync.dma_start(ident_sb[:], ident.ap())
                    nc.sync.dma_start(ones_col[:], onesc.ap())
                    nc.sync.dma_start(ones_row[:], onesr.ap())
                elif stage == 1:    # attention needs these
                    nc.sync.dma_start(em_sb[:], em2.ap())
                    nc.sync.dma_start(wo_sb[:], wo.ap())

            for b in range(B):
                with ExitStack() as bctx:
                    bpool = bctx.enter_context(
                        tc.tile_pool(name=f"b{b}_persist", bufs=1))
                    K_cks = [bpool.tile([128, SC], F16, name=f"K_sb{b}_{s_}")
                             for s_ in range(NSC)]
                    V_cks = [bpool.tile([128, SC // 128, 128], F16,
                                        name=f"V_sb{b}_{s_}")
                             for s_ in range(NSC)]
                    Q_cks = [[bpool.tile([128, SC], F16,
                                         name=f"Q_sb{b}_{i}_{s_}")
                              for s_ in range(NSC)] for i in range(QH)]

                    # ---------------- QKV projection + RoPE ----------------
                    with ExitStack() as pctx:
                        pps = pctx.enter_context(
                            tc.tile_pool(name=f"b{b}_qkv_ps", bufs=1,
                                         space="PSUM"))
                        sp = pctx.enter_context(
                            tc.tile_pool(name=f"b{b}_qkv_sb", bufs=1))

                        for sc in range(NSC):
                            cs = slice(sc * SC, (sc + 1) * SC)
                            xgs = get_xgroups(b, sc)
                            if b == 0 and sc == 0:
                                late_preloads(0)
                            elif b == 0 and sc == NSC - 1:
                                late_preloads(1)

                            accs = [pps.tile([128, SC], F32, tag="qkvacc",
                                             bufs=QH + 2, name=f"acc{j}")
                                    for j in range(QH + 2)]
                            for ct in range(CT):
                                xr = xgs[ct]
                                st, sp_ = (ct == 0), (ct == CT - 1)
                                for j in range(QH):
                                    nc.tensor.matmul(
                                        accs[j][:],
                                        wq_sb[:, ct, j * HD:(j + 1) * HD],
                                        xr, start=st, stop=sp_)
                                nc.tensor.matmul(accs[QH][:], wk_sb[:, ct, :],
                                                 xr, start=st, stop=sp_)
                                nc.tensor.matmul(accs[QH + 1][:],
                                                 wv_sb[:, ct, :],
                                                 xr, start=st, stop=sp_)

                            # V first (fp16): copy out of PSUM, PE-transpose
                            # to [s, d]. Emitted before the RoPE copies so
                            # the PE transposes aren't queued behind them
                            # on the scalar engine at the phase tail.
                            vstage = sp.tile([128, SC], F16, tag="vst", bufs=2,
                                             name="vstage")
                            nc.scalar.copy(vstage[:], accs[QH + 1][:])
                            for j in range(SC // 128):
                                v_ps = pps.tile([128, 128], F16, tag="vtr",
                                                bufs=2, name="v_ps")
                                nc.tensor.transpose(
                                    v_ps[:], vstage[:, j * 128:(j + 1) * 128],
                                    ident_sb[:])
                                nc.vector.tensor_copy(
                                    V_cks[sc][:, j, :], v_ps[:])

                            # RoPE on the QH q-heads and the k head (fp16).
                            rope_dsts = [q_ck[sc][:] for q_ck in Q_cks]
                            rope_dsts.append(K_cks[sc][:])
                            for j, dst in enumerate(rope_dsts):
                                t_ps = accs[j]
                                t_sb = sp.tile([128, SC], F16, tag="tsb",
                                               bufs=5, name="t_sb")
                                nc.scalar.copy(t_sb[:], t_ps[:])
                                rot_sb = sp.tile([128, SC], F16, tag="rot",
                                                 bufs=4, name="rot_sb")
                                # rotate-half via partition-shifted DMA;
                                # sign of the first half folded into sinT.
                                nc.sync.dma_start(rot_sb[0:64, :],
                                                  t_sb[64:128, :])
                                nc.sync.dma_start(rot_sb[64:128, :],
                                                  t_sb[0:64, :])
                                tmp1 = sp.tile([128, SC], F16, tag="tmp1",
                                               bufs=4, name="tmp1")
                                nc.vector.tensor_mul(tmp1[:], t_sb[:],
                                                     cos_sb[:, cs])
                                tmp2 = sp.tile([128, SC], F16, tag="tmp2",
                                               bufs=4, name="tmp2")
                                nc.vector.tensor_mul(tmp2[:], rot_sb[:],
                                                     sin_sb[:, cs])
                                nc.vector.tensor_add(dst, tmp1[:], tmp2[:])

                    # ------------- attention + out-projection -------------
                    if b + 1 < B:
                        prefetch_xgroups(b + 1, 0)  # next batch's x
                    with ExitStack() as actx:
                        aps = actx.enter_context(
                            tc.tile_pool(name=f"b{b}_attn_ps", bufs=1,
                                         space="PSUM"))
                        asb = actx.enter_context(
                            tc.tile_pool(name=f"b{b}_attn_sb", bufs=1))

                        # Pending out-projection work, drained into the next
                        # q-chunk's (ACT-bound) score/AV loop.
                        pending = []  # list of thunks, one per mt unit
                        # Slot-deferred emissions (head-end normalization
                        # chains) so PE steps never head-of-line block.
                        sched = []    # [slots_left, thunk]

                        def defer(n, fn):
                            sched.append([n, fn])

                        def tick():
                            for item in sched[:]:
                                item[0] -= 1
                                if item[0] <= 0:
                                    sched.remove(item)
                                    item[1]()

                        def emit_oproj(qc, ohs):
                            qs = slice(qc * SC, (qc + 1) * SC)

                            def unit(mt, qs=qs, ohs=ohs):
                                y_ps = aps.tile([128, SC], F32, tag="y",
                                                bufs=2, name="y_ps")
                                for hh in range(QH):
                                    nc.tensor.matmul(
                                        y_ps[:],
                                        wo_sb[:, hh, mt * 128:(mt + 1) * 128],
                                        ohs[hh][:],
                                        start=(hh == 0), stop=(hh == QH - 1))
                                y_sb = asb.tile([128, SC], F16, tag="yout",
                                                bufs=5, name="y_sb")
                                if mt % 2 == 0:
                                    nc.vector.tensor_copy(y_sb[:], y_ps[:])
                                else:
                                    nc.scalar.copy(y_sb[:], y_ps[:])
                                nc.sync.dma_start(
                                    yT_ap[b, mt * 128:(mt + 1) * 128, qs],
                                    y_sb[:])
                            for mt in range(CT):
                                pending.append(lambda mt=mt: unit(mt))

                        def drain(n):
                            for _ in range(min(n, len(pending))):
                                pending.pop(0)()

                        for qc in range(NSC):
                            npair = 2 * qc + 2   # kt pairs; last 2 masked
                            # pair-slots in this qc (3 heads); hold off
                            # draining for the first few so the previous
                            # q-chunk's oh normalization latency is hidden.
                            slots = 3 * npair
                            hold = 5
                            for hh in range(QH):
                                av_ps = aps.tile([128, SC], F32, tag="av",
                                                 bufs=2, name="av_ps")
                                E2_acc = asb.tile([128, 2, SC], F16,
                                                  tag="eacc", bufs=2,
                                                  name="E2_acc")
                                for pi in range(npair):
                                    kt0 = 2 * pi
                                    masked = pi >= npair - 2
                                    s2 = aps.tile([128, 2, SC], F32, tag="s2",
                                                  bufs=2, name="s2")
                                    for j in range(2):
                                        kb, kj = divmod(kt0 + j, SC // 128)
                                        nc.tensor.matmul(
                                            s2[:, j, :],
                                            K_cks[kb][:, kj * 128:
                                                      (kj + 1) * 128],
                                            Q_cks[hh][qc][:],
                                            start=True, stop=True)
                                    if pi == 0:
                                        e2 = E2_acc  # exp lands in the accum
                                    else:
                                        e2 = asb.tile([128, 2, SC], F16,
                                                      tag="e2", bufs=4,
                                                      name="e2")
                                    if masked:
                                        e_st = asb.tile([128, 2, SC], F16,
                                                        tag="est", bufs=2,
                                                        name="e_st")
                                        nc.scalar.activation(
                                            e_st[:], s2[:], AFT.Exp,
                                            scale=SCALE)
                                        mp = 2 * qc + (pi - (npair - 2))
                                        nc.vector.tensor_mul(
                                            e2[:], e_st[:],
                                            em_sb[:, mp, :, :])
                                    else:
                                        nc.scalar.activation(
                                            e2[:], s2[:], AFT.Exp,
                                            scale=SCALE)
                                    for j in range(2):
                                        kb, kj = divmod(kt0 + j, SC // 128)
                                        nc.tensor.matmul(
                                            av_ps[:], V_cks[kb][:, kj, :],
                                            e2[:, j, :],
                                            start=(pi == 0 and j == 0),
                                            stop=(pi == npair - 1 and j == 1))
                                    if pi > 0:
                                        nc.vector.tensor_add(
                                            E2_acc[:], E2_acc[:], e2[:])
                                    # overlap pending out-proj with this
                                    # ACT-bound loop
                                    tick()
                                    if pending and hold <= 0:
                                        drain(-(-len(pending) // slots))
                                    slots -= 1
                                    hold -= 1

                                # Head-end softmax normalization: rowsum via
                                # ones-matmuls (PSUM banks borrowed from the
                                # y tag), reciprocal, row-broadcast matmul,
                                # normalize. Each PE step is deferred by
                                # pair-slots so it lands in the stream only
                                # after its inputs are surely ready.
                                oh = asb.tile([128, SC], F16, tag="oh",
                                              bufs=2 * QH + 1, name="oh")
                                if hh == 0:
                                    ohs = []
                                ohs.append(oh)

                                def chain(E2_acc=E2_acc, av_ps=av_ps, oh=oh):
                                    r_t = aps.tile([128, SC], F32, tag="y",
                                                   bufs=2, name="r_ps")
                                    invf = asb.tile([1, SC], F32, tag="invf",
                                                    bufs=2, name="invf")
                                    inv = asb.tile([1, SC], F16, tag="inv",
                                                   bufs=2, name="inv")
                                    invb = asb.tile([128, SC], F32, tag="rb",
                                                    bufs=2, name="invb")

                                    def st_r():
                                        nc.tensor.matmul(
                                            r_t[0:1, :], ones_col[:],
                                            E2_acc[:, 0, :],
                                            start=True, stop=False)
                                        nc.tensor.matmul(
                                            r_t[0:1, :], ones_col[:],
                                            E2_acc[:, 1, :],
                                            start=False, stop=True)

                                    def st_recip():
                                        nc.vector.reciprocal_approx_fast(
                                            invf[:], r_t[0:1, :])
                                        nc.vector.tensor_copy(inv[:], invf[:])

                                    def st_invb():
                                        invb_t = aps.tile(
                                            [128, SC], F32, tag="y", bufs=2,
                                            name="invb_ps")
                                        nc.tensor.matmul(
                                            invb_t[:], ones_row[:], inv[:],
                                            start=True, stop=True)
                                        nc.scalar.copy(invb[:], invb_t[:])

                                    def st_oh():
                                        nc.vector.tensor_mul(
                                            oh[:], av_ps[:], invb[:])

                                    return [st_r, st_recip, st_invb, st_oh]

                                for i, st in enumerate(chain()):
                                    defer(i + 1, st)

                            emit_oproj(qc, ohs)

                        while sched:
                            tick()
                        drain(len(pending))  # last q-chunk's out-proj

    nc.compile()
    return nc


def make_inputs(x, freqs_cos, freqs_sin, mask, wq, wk, wv, wo):
    """Host-side preprocessing -> per-core input maps (repacked layouts)."""
    f32, f16 = np.float32, np.float16
    x = np.asarray(x, f32)
    xT = np.transpose(x, (0, 2, 1)).astype(f16)          # [B, D, S]
    # -> [B, NSC, 128, CT, SC]: fat contiguous per-partition DMA lines
    xTr = np.ascontiguousarray(
        xT.reshape(B, CT, 128, NSC, SC).transpose(0, 3, 2, 1, 4))
    cosT = np.ascontiguousarray(
        np.concatenate([freqs_cos, freqs_cos], axis=1).T.astype(f16))
    sinT = np.concatenate([freqs_sin, freqs_sin], axis=1).T.astype(f32).copy()
    sinT[:HD // 2] *= -1.0  # sign of rotate-half folded in
    sinT = np.ascontiguousarray(sinT.astype(f16))

    em = np.exp(np.asarray(mask, f32)[0, 0]).T  # [k, q] multiplicative
    em2 = np.zeros((2 * NSC, 128, 2, SC), f16)
    for qc in range(NSC):
        for p in range(2):
            for j in range(2):
                kt = 4 * qc + 2 * p + j
                em2[2 * qc + p, :, j, :] = em[
                    kt * 128:(kt + 1) * 128, qc * SC:(qc + 1) * SC]
    em2r = np.ascontiguousarray(em2.transpose(1, 0, 2, 3))
    identity = np.ascontiguousarray(np.eye(128, dtype=f16))

    wqT = np.asarray(wq, f32).T.astype(f16)
    wkT = np.asarray(wk, f32).T.astype(f16)
    wvT = np.asarray(wv, f32).T.astype(f16)
    woT = np.asarray(wo, f32).T.astype(f16)

    def pack_w(wt):  # [D, m] -> [128, CT, m]
        return np.ascontiguousarray(
            wt.reshape(-1, 128, wt.shape[1]).transpose(1, 0, 2))

    in_maps = []
    for h in range(N_CORES):
        qsl = slice(h * QH * HD, (h + 1) * QH * HD)
        ksl = slice(h * HD, (h + 1) * HD)
        in_maps.append({
            "xT": xTr,
            "cosT": cosT,
            "sinT": sinT,
            "wq": pack_w(wqT[:, qsl]),
            "wk": pack_w(wkT[:, ksl]),
            "wv": pack_w(wvT[:, ksl]),
            "wo": pack_w(woT[qsl, :]),   # [384, D] -> [128, QH, D]
            "em2": em2r,
            "ident": identity,
            "onesc": np.ones((128, 1), f16),
            "onesr": np.ones((1, 128), f16),
        })
    return in_maps


_CACHE = {}


def kernel(x, freqs_cos, freqs_sin, mask, wq, wk, wv, wo):
    global LAST_EXEC_NS, LAST_RESULTS
    assert tuple(x.shape) == (B, S, D), x.shape

    in_maps = make_inputs(x, freqs_cos, freqs_sin, mask, wq, wk, wv, wo)

    if "prog" not in _CACHE:
        _CACHE["prog"] = build_program()
    nc = _CACHE["prog"]

    kwargs = {}
    if TRACE:
        kwargs = dict(trace=True, trace_cores=[0])
    res = run_bass_kernel_spmd(nc, in_maps, list(range(N_CORES)), **kwargs)
    LAST_EXEC_NS = res.exec_time_ns
    LAST_RESULTS = res

    acc = np.zeros((B, D, S), np.float32)
    for i in range(N_CORES):
        acc += res.results[i]["yT"].astype(np.float32)
    y = np.ascontiguousarray(np.transpose(acc, (0, 2, 1)).astype(np.float32))
    return y


# revision 39
# speedup vs baseline: 1.1892x; 1.1892x over previous
"""GQA attention kernel for Trainium2 (8 NeuronCores, Bass/Tile).

Problem: B=2, S=2048, D=3072, 24 Q heads / 8 KV heads, HD=128, RoPE,
additive causal mask, softmax, output projection.

Sharding: tensor-parallel over heads. Core h owns KV head h and Q heads
{3h, 3h+1, 3h+2} for BOTH batch elements. Each core produces a partial
y^T = wo_slice^T.T @ attn_out_heads^T of shape (B, D, S) in fp16; the
host sums the 8 partials in fp32 and transposes back.

Layout: everything transposed ([feature, token]) on chip so every
matmul contracts on the partition dim with a 512-wide fp16 moving
operand (1 cycle/row on the PE):
  - x^T streamed from DRAM (host pre-transposed, fp16)
  - QKV projection -> Q^T,K^T [hd, S] per head; RoPE in transposed
    layout (rotate-half via partition-shifted SBUF DMA, sign folded
    into sinT)
  - scores^T [k, q] = K-tile @ Q^T into PAIRED PSUM banks; ONE exp per
    pair on ACT ([128,2,512]) with the 1/sqrt(HD) scale folded in;
    causal mask applied multiplicatively (fp16, DVE 4x mode)
  - attn@V accumulated in PSUM; softmax denominator: DVE accumulates
    exp pairs (fp16), then a slot-DEFERRED ones-matmul rowsum +
    reciprocal + ones-row broadcast matmul normalize without ever
    head-of-line blocking the PE
  - out-projection matmuls interleaved into the NEXT q-chunk's
    score/AV loop so the ACT-bound k-loop and PE-bound out-proj
    overlap; host-repacked dram layouts give every DMA fat contiguous
    lines, with first-chunk weight/x groups interleaved for cold start
"""

import math
import os
import sys

import numpy as np

for _p in ("/opt/trn_rl_repo",):
    if os.path.isdir(_p) and _p not in sys.path:
        sys.path.insert(0, _p)

import concourse.bass as bass  # noqa: E402
import concourse.bass_isa as bass_isa  # noqa: E402
import concourse.mybir as mybir  # noqa: E402
import concourse.tile as tile  # noqa: E402
from concourse import bacc  # noqa: E402
from concourse.bass_utils import run_bass_kernel_spmd  # noqa: E402

F32 = mybir.dt.float32
F16 = mybir.dt.float16
AFT = mybir.ActivationFunctionType

N_CORES = 8

# Set by test harness to capture a profile on the next kernel() call.
TRACE = False
LAST_EXEC_NS = None
LAST_RESULTS = None

B, S, D = 2, 2048, 3072
QH, HD, SC = 3, 128, 512
CT = D // 128          # 24 contraction tiles
KT = S // 128          # 16 key tiles
NSC = S // SC          # 4 token chunks
XG = 6                 # x tiles per DMA group
SCALE = 1.0 / math.sqrt(HD)


def build_program():
    nc = bacc.Bacc("TRN2", target_bir_lowering=False, debug=False,
                   num_devices=N_CORES)

    # All dram layouts host-repacked so every DMA has long contiguous
    # per-partition lines (fat packets) and needs no on-the-fly rearrange.
    xT = nc.declare_dram_parameter("xT", [B, NSC, 128, CT, SC], F16,
                                   isOutput=False)
    cosT = nc.declare_dram_parameter("cosT", [HD, S], F16, isOutput=False)
    sinT = nc.declare_dram_parameter("sinT", [HD, S], F16, isOutput=False)
    onesc = nc.declare_dram_parameter("onesc", [128, 1], F16, isOutput=False)
    onesr = nc.declare_dram_parameter("onesr", [1, 128], F16, isOutput=False)
    wq = nc.declare_dram_parameter("wq", [128, CT, QH * HD], F16,
                                   isOutput=False)
    wk = nc.declare_dram_parameter("wk", [128, CT, HD], F16, isOutput=False)
    wv = nc.declare_dram_parameter("wv", [128, CT, HD], F16, isOutput=False)
    wo = nc.declare_dram_parameter("wo", [128, QH, D], F16, isOutput=False)
    # 8 masked (qc, kt-pair) blocks of exp(mask)^T, fp16 {0,1}
    em2 = nc.declare_dram_parameter("em2", [128, 2 * NSC, 2, SC], F16,
                                    isOutput=False)
    ident = nc.declare_dram_parameter("ident", [128, 128], F16, isOutput=False)
    yT = nc.declare_dram_parameter("yT", [B, D, S], F16, isOutput=True)

    xT_ap, yT_ap = xT.ap(), yT.ap()

    with tile.TileContext(nc) as tc:
        from contextlib import ExitStack
        with ExitStack() as top:
            const = top.enter_context(tc.tile_pool(name="const", bufs=1))
            stream = top.enter_context(tc.tile_pool(name="stream", bufs=1))

            wq_sb = const.tile([128, CT, QH * HD], F16, name="wq_sb")
            wk_sb = const.tile([128, CT, HD], F16, name="wk_sb")
            wv_sb = const.tile([128, CT, HD], F16, name="wv_sb")
            wo_sb = const.tile([128, QH, D], F16, name="wo_sb")
            cos_sb = const.tile([128, S], F16, name="cos_sb")
            sin_sb = const.tile([128, S], F16, name="sin_sb")
            em_sb = const.tile([128, 2 * NSC, 2, SC], F16, name="em_sb")
            ident_sb = const.tile([128, 128], F16, name="ident_sb")
            ones_col = const.tile([128, 1], F16, name="ones_col")
            ones_row = const.tile([1, 128], F16, name="ones_row")

            # x chunk group tiles, cached so the next batch's first chunk
            # can be prefetched during the previous batch's attention.
            xg_cache = {}

            def prefetch_xgroups(b, sc, with_weights=False):
                key = (b, sc)
                if key not in xg_cache:
                    # Cold start uses fine 3-ct groups so the first matmuls
                    # gate on ~0.8MB of DMA instead of several MB.
                    plan = [3] * 8 if with_weights else [XG] * (CT // XG)
                    xgs, ct0 = [], 0
                    for n in plan:
                        gs = slice(ct0, ct0 + n)
                        if with_weights:
                            # rate-match weight and x arrival per group
                            nc.sync.dma_start(wq_sb[:, gs, :], wq.ap()[:, gs])
                            nc.sync.dma_start(wk_sb[:, gs, :], wk.ap()[:, gs])
                            nc.sync.dma_start(wv_sb[:, gs, :], wv.ap()[:, gs])
                        xg = stream.tile([128, n, SC], F16, tag="xg",
                                         bufs=4, name="xg",
                                         padded_shape=[128, XG, SC])
                        nc.sync.dma_start(xg[:], xT_ap[b, sc, :, gs, :])
                        xgs.append((xg, ct0, n))
                        ct0 += n
                    xg_cache[key] = xgs

            def get_xgroups(b, sc):
                prefetch_xgroups(b, sc)
                groups = xg_cache.pop((b, sc))
                xr_map = {}
                for xg, ct0, n in groups:
                    for k in range(n):
                        xr_map[ct0 + k] = xg[:, k, :]
                return xr_map

            prefetch_xgroups(0, 0, with_weights=True)

            def late_preloads(stage):
                # Staged behind the x chunks that precede their first use so
                # they never delay the proj matmul stream.
                if stage == 0:      # RoPE needs these from chunk 0
                    nc.sync.dma_start(cos_sb[:], cosT.ap())
                    nc.sync.dma_start(sin_sb[:], sinT.ap())
                    nc.sync.dma_start(ident_sb[:], ident.ap())
                    nc.sync.dma_start(ones_col[:], onesc.ap())
                    nc.sync.dma_start(ones_row[:], onesr.ap())
                elif stage == 1:    # attention needs these
                    nc.sync.dma_start(em_sb[:], em2.ap())
                    nc.sync.dma_start(wo_sb[:], wo.ap())

            for b in range(B):
                with ExitStack() as bctx:
                    bpool = bctx.enter_context(
                        tc.tile_pool(name=f"b{b}_persist", bufs=1))
                    K_cks = [bpool.tile([128, SC], F16, name=f"K_sb{b}_{s_}")
                             for s_ in range(NSC)]
                    V_cks = [bpool.tile([128, SC // 128, 128], F16,
                                        name=f"V_sb{b}_{s_}")
                             for s_ in range(NSC)]
                    Q_cks = [[bpool.tile([128, SC], F16,
                                         name=f"Q_sb{b}_{i}_{s_}")
                              for s_ in range(NSC)] for i in range(QH)]

                    # ---------------- QKV projection + RoPE ----------------
                    with ExitStack() as pctx:
                        pps = pctx.enter_context(
                            tc.tile_pool(name=f"b{b}_qkv_ps", bufs=1,
                                         space="PSUM"))
                        sp = pctx.enter_context(
                            tc.tile_pool(name=f"b{b}_qkv_sb", bufs=1))

                        for sc in range(NSC):
                            cs = slice(sc * SC, (sc + 1) * SC)
                            xgs = get_xgroups(b, sc)
                            if b == 0 and sc == 0:
                                late_preloads(0)
                            elif b == 0 and sc == NSC - 1:
                                late_preloads(1)

                            accs = [pps.tile([128, SC], F32, tag="qkvacc",
                                             bufs=QH + 2, name=f"acc{j}")
                                    for j in range(QH + 2)]
                            for ct in range(CT):
                                xr = xgs[ct]
                                st, sp_ = (ct == 0), (ct == CT - 1)
                                for j in range(QH):
                                    nc.tensor.matmul(
                                        accs[j][:],
                                        wq_sb[:, ct, j * HD:(j + 1) * HD],
                                        xr, start=st, stop=sp_)
                                nc.tensor.matmul(accs[QH][:], wk_sb[:, ct, :],
                                                 xr, start=st, stop=sp_)
                                nc.tensor.matmul(accs[QH + 1][:],
                                                 wv_sb[:, ct, :],
                                                 xr, start=st, stop=sp_)

                            # V first (fp16): copy out of PSUM, PE-transpose
                            # to [s, d]. Emitted before the RoPE copies so
                            # the PE transposes aren't queued behind them
                            # on the scalar engine at the phase tail.
                            vstage = sp.tile([128, SC], F16, tag="vst", bufs=2,
                                             name="vstage")
                            nc.scalar.copy(vstage[:], accs[QH + 1][:])
                            for j in range(SC // 128):
                                v_ps = pps.tile([128, 128], F16, tag="vtr",
                                                bufs=2, name="v_ps")
                                nc.tensor.transpose(
                                    v_ps[:], vstage[:, j * 128:(j + 1) * 128],
                                    ident_sb[:])
                                nc.vector.tensor_copy(
                                    V_cks[sc][:, j, :], v_ps[:])

                            # RoPE on the QH q-heads and the k head (fp16).
                            rope_dsts = [q_ck[sc][:] for q_ck in Q_cks]
                            rope_dsts.append(K_cks[sc][:])
                            for j, dst in enumerate(rope_dsts):
                                t_ps = accs[j]
                                t_sb = sp.tile([128, SC], F16, tag="tsb",
                                               bufs=5, name="t_sb")
                                nc.scalar.copy(t_sb[:], t_ps[:])
                                rot_sb = sp.tile([128, SC], F16, tag="rot",
                                                 bufs=4, name="rot_sb")
                                # rotate-half via partition-shifted DMA;
                                # sign of the first half folded into sinT.
                                nc.sync.dma_start(rot_sb[0:64, :],
                                                  t_sb[64:128, :])
                                nc.sync.dma_start(rot_sb[64:128, :],
                                                  t_sb[0:64, :])
                                tmp1 = sp.tile([128, SC], F16, tag="tmp1",
                                               bufs=4, name="tmp1")
                                nc.vector.tensor_mul(tmp1[:], t_sb[:],
                                                     cos_sb[:, cs])
                                tmp2 = sp.tile([128, SC], F16, tag="tmp2",
                                               bufs=4, name="tmp2")
                                nc.vector.tensor_mul(tmp2[:], rot_sb[:],
                                                     sin_sb[:, cs])
                                nc.vector.tensor_add(dst, tmp1[:], tmp2[:])

                    # ------------- attention + out-projection -------------
                    if b + 1 < B:
                        prefetch_xgroups(b + 1, 0)  # next batch's x
                    with ExitStack() as actx:
                        aps = actx.enter_context(
                            tc.tile_pool(name=f"b{b}_attn_ps", bufs=1,
                                         space="PSUM"))
                        asb = actx.enter_context(
                            tc.tile_pool(name=f"b{b}_attn_sb", bufs=1))

                        # Pending out-projection work, drained into the next
                        # q-chunk's (ACT-bound) score/AV loop.
                        pending = []  # list of thunks, one per mt unit
                        # Slot-deferred emissions (head-end normalization
                        # chains) so PE steps never head-of-line block.
                        sched = []    # [slots_left, thunk]

                        def defer(n, fn):
                            sched.append([n, fn])

                        def tick():
                            for item in sched[:]:
                                item[0] -= 1
                                if item[0] <= 0:
                                    sched.remove(item)
                                    item[1]()

                        def emit_oproj(qc, ohs):
                            qs = slice(qc * SC, (qc + 1) * SC)

                            def unit(mt, qs=qs, ohs=ohs):
                                y_ps = aps.tile([128, SC], F32, tag="y",
                                                bufs=2, name="y_ps")
                                for hh in range(QH):
                                    nc.tensor.matmul(
                                        y_ps[:],
                                        wo_sb[:, hh, mt * 128:(mt + 1) * 128],
                                        ohs[hh][:],
                                        start=(hh == 0), stop=(hh == QH - 1))
                                y_sb = asb.tile([128, SC], F16, tag="yout",
                                                bufs=5, name="y_sb")
                                if mt % 2 == 0:
                                    nc.vector.tensor_copy(y_sb[:], y_ps[:])
                                else:
                                    nc.scalar.copy(y_sb[:], y_ps[:])
                                nc.sync.dma_start(
                                    yT_ap[b, mt * 128:(mt + 1) * 128, qs],
                                    y_sb[:])
                            for mt in range(CT):
                                pending.append(lambda mt=mt: unit(mt))

                        def drain(n):
                            for _ in range(min(n, len(pending))):
                                pending.pop(0)()

                        for qc in range(NSC):
                            npair = 2 * qc + 2   # kt pairs; last 2 masked
                            # pair-slots in this qc (3 heads); hold off
                            # draining for the first few so the previous
                            # q-chunk's oh normalization latency is hidden.
                            slots = 3 * npair
                            hold = 5
                            for hh in range(QH):
                                av_ps = aps.tile([128, SC], F32, tag="av",
                                                 bufs=2, name="av_ps")
                                E2_acc = asb.tile([128, 2, SC], F16,
                                                  tag="eacc", bufs=2,
                                                  name="E2_acc")
                                for pi in range(npair):
                                    kt0 = 2 * pi
                                    masked = pi >= npair - 2
                                    s2 = aps.tile([128, 2, SC], F32, tag="s2",
                                                  bufs=2, name="s2")
                                    for j in range(2):
                                        kb, kj = divmod(kt0 + j, SC // 128)
                                        nc.tensor.matmul(
                                            s2[:, j, :],
                                            K_cks[kb][:, kj * 128:
                                                      (kj + 1) * 128],
                                            Q_cks[hh][qc][:],
                                            start=True, stop=True)
                                    if pi == 0:
                                        e2 = E2_acc  # exp lands in the accum
                                    else:
                                        e2 = asb.tile([128, 2, SC], F16,
                                                      tag="e2", bufs=4,
                                                      name="e2")
                                    if masked:
                                        e_st = asb.tile([128, 2, SC], F16,
                                                        tag="est", bufs=2,
                                                        name="e_st")
                                        nc.scalar.activation(
                                            e_st[:], s2[:], AFT.Exp,
                                            scale=SCALE)
                                        mp = 2 * qc + (pi - (npair - 2))
                                        nc.vector.tensor_mul(
                                            e2[:], e_st[:],
                                            em_sb[:, mp, :, :])
                                    else:
                                        nc.scalar.activation(
                                            e2[:], s2[:], AFT.Exp,
                                            scale=SCALE)
                                    for j in range(2):
                                        kb, kj = divmod(kt0 + j, SC // 128)
                                        nc.tensor.matmul(
                                            av_ps[:], V_cks[kb][:, kj, :],
                                            e2[:, j, :],
                                            start=(pi == 0 and j == 0),
                                            stop=(pi == npair - 1 and j == 1))
                                    if pi > 0:
                                        nc.vector.tensor_add(
                                            E2_acc[:], E2_acc[:], e2[:])
                                    # overlap pending out-proj with this
                                    # ACT-bound loop
                                    tick()
                                    if pending and hold <= 0:
                                        drain(-(-len(pending) // slots))
                                    slots -= 1
                                    hold -= 1

                                # Head-end softmax normalization: rowsum via
                                # ones-matmuls (PSUM banks borrowed from the
                                # y tag), reciprocal, row-broadcast matmul,
                                # normalize. Each PE step is deferred by
                                # pair-slots so it lands in the stream only
                                # after its inputs are surely ready.
                                oh = asb.tile([128, SC], F16, tag="oh",
                                              bufs=2 * QH + 1, name="oh")
                                if hh == 0:
                                    ohs = []
                                ohs.append(oh)

                                def chain(E2_acc=E2_acc, av_ps=av_ps, oh=oh):
                                    r_t = aps.tile([128, SC], F32, tag="y",
                                                   bufs=2, name="r_ps")
                                    invf = asb.tile([1, SC], F32, tag="invf",
                                                    bufs=2, name="invf")
                                    inv = asb.tile([1, SC], F16, tag="inv",
                                                   bufs=2, name="inv")
                                    invb = asb.tile([128, SC], F32, tag="rb",
                                                    bufs=2, name="invb")

                                    def st_r():
                                        nc.tensor.matmul(
                                            r_t[0:1, :], ones_col[:],
                                            E2_acc[:, 0, :],
                                            start=True, stop=False)
                                        nc.tensor.matmul(
                                            r_t[0:1, :], ones_col[:],
                                            E2_acc[:, 1, :],
                                            start=False, stop=True)

                                    def st_recip():
                                        nc.vector.reciprocal_approx_fast(
                                            invf[:], r_t[0:1, :])
                                        nc.vector.tensor_copy(inv[:], invf[:])

                                    def st_invb():
                                        invb_t = aps.tile(
                                            [128, SC], F32, tag="y", bufs=2,
                                            name="invb_ps")
                                        nc.tensor.matmul(
                                            invb_t[:], ones_row[:], inv[:],
                                            start=True, stop=True)
                                        nc.scalar.copy(invb[:], invb_t[:])

                                    def st_oh():
                                        nc.vector.tensor_mul(
                                            oh[:], av_ps[:], invb[:])

                                    return [st_r, st_recip, st_invb, st_oh]

                                for i, st in enumerate(chain()):
                                    defer(i + 1, st)

                            emit_oproj(qc, ohs)

                        while sched:
                            tick()
                        drain(len(pending))  # last q-chunk's out-proj

    nc.compile()
    return nc


def make_inputs(x, freqs_cos, freqs_sin, mask, wq, wk, wv, wo):
    """Host-side preprocessing -> per-core input maps (repacked layouts)."""
    f32, f16 = np.float32, np.float16
    x = np.asarray(x, f32)
    xT = np.transpose(x, (0, 2, 1)).astype(f16)          # [B, D, S]
    # -> [B, NSC, 128, CT, SC]: fat contiguous per-partition DMA lines
    xTr = np.ascontiguousarray(
        xT.reshape(B, CT, 128, NSC, SC).transpose(0, 3, 2, 1, 4))
    cosT = np.ascontiguousarray(
        np.concatenate([freqs_cos, freqs_cos], axis=1).T.astype(f16))
    sinT = np.concatenate([freqs_sin, freqs_sin], axis=1).T.astype(f32).copy()
    sinT[:HD // 2] *= -1.0  # sign of rotate-half folded in
    sinT = np.ascontiguousarray(sinT.astype(f16))

    em = np.exp(np.asarray(mask, f32)[0, 0]).T  # [k, q] multiplicative
    em2 = np.zeros((2 * NSC, 128, 2, SC), f16)
    for qc in range(NSC):
        for p in range(2):
            for j in range(2):
                kt = 4 * qc + 2 * p + j
                em2[2 * qc + p, :, j, :] = em[
                    kt * 128:(kt + 1) * 128, qc * SC:(qc + 1) * SC]
    em2r = np.ascontiguousarray(em2.transpose(1, 0, 2, 3))
    identity = np.ascontiguousarray(np.eye(128, dtype=f16))

    wqT = np.asarray(wq, f32).T.astype(f16)
    wkT = np.asarray(wk, f32).T.astype(f16)
    wvT = np.asarray(wv, f32).T.astype(f16)
    woT = np.asarray(wo, f32).T.astype(f16)

    def pack_w(wt):  # [D, m] -> [128, CT, m]
        return np.ascontiguousarray(
            wt.reshape(-1, 128, wt.shape[1]).transpose(1, 0, 2))

    in_maps = []
    for h in range(N_CORES):
        qsl = slice(h * QH * HD, (h + 1) * QH * HD)
        ksl = slice(h * HD, (h + 1) * HD)
        in_maps.append({
            "xT": xTr,
            "cosT": cosT,
            "sinT": sinT,
            "wq": pack_w(wqT[:, qsl]),
            "wk": pack_w(wkT[:, ksl]),
            "wv": pack_w(wvT[:, ksl]),
            "wo": pack_w(woT[qsl, :]),   # [384, D] -> [128, QH, D]
            "em2": em2r,
            "ident": identity,
            "onesc": np.ones((128, 1), f16),
            "onesr": np.ones((1, 128), f16),
        })
    return in_maps


_CACHE = {}


def kernel(x, freqs_cos, freqs_sin, mask, wq, wk, wv, wo):
    global LAST_EXEC_NS, LAST_RESULTS
    assert tuple(x.shape) == (B, S, D), x.shape

    in_maps = make_inputs(x, freqs_cos, freqs_sin, mask, wq, wk, wv, wo)

    if "prog" not in _CACHE:
        _CACHE["prog"] = build_program()
    nc = _CACHE["prog"]

    kwargs = {}
    if TRACE:
        kwargs = dict(trace=True, trace_cores=[0])
    res = run_bass_kernel_spmd(nc, in_maps, list(range(N_CORES)), **kwargs)
    LAST_EXEC_NS = res.exec_time_ns
    LAST_RESULTS = res

    acc = np.zeros((B, D, S), np.float32)
    for i in range(N_CORES):
        acc += res.results[i]["yT"].astype(np.float32)
    y = np.ascontiguousarray(np.transpose(acc, (0, 2, 1)).astype(np.float32))
    return y


# revision 42
# speedup vs baseline: 1.2149x; 1.0216x over previous
"""GQA attention kernel for Trainium2 (8 NeuronCores, Bass/Tile).

Problem: B=2, S=2048, D=3072, 24 Q heads / 8 KV heads, HD=128, RoPE,
additive causal mask, softmax, output projection.

Sharding: tensor-parallel over heads. Core h owns KV head h and Q heads
{3h, 3h+1, 3h+2} for BOTH batch elements. Each core produces a partial
y^T = wo_slice^T.T @ attn_out_heads^T of shape (B, D, S) in fp16; the
host sums the 8 partials in fp32 and transposes back.

Layout: everything transposed ([feature, token]) on chip so every
matmul contracts on the partition dim with a 512-wide fp16 moving
operand (1 cycle/row on the PE):
  - x^T streamed from DRAM (host pre-transposed, fp16)
  - QKV projection -> Q^T,K^T [hd, S] per head; RoPE in transposed
    layout (rotate-half via partition-shifted SBUF DMA, sign folded
    into sinT)
  - scores^T [k, q] = K-tile @ Q^T into PAIRED PSUM banks; ONE exp per
    pair on ACT ([128,2,512]) with the 1/sqrt(HD) scale folded in;
    causal mask applied multiplicatively (fp16, DVE 4x mode)
  - attn@V accumulated in PSUM; softmax denominator: DVE accumulates
    exp pairs (fp16), then a slot-DEFERRED ones-matmul rowsum +
    reciprocal + ones-row broadcast matmul normalize without ever
    head-of-line blocking the PE
  - out-projection matmuls interleaved into the NEXT q-chunk's
    score/AV loop so the ACT-bound k-loop and PE-bound out-proj
    overlap; host-repacked dram layouts give every DMA fat contiguous
    lines, with first-chunk weight/x groups interleaved for cold start
"""

import math
import os
import sys

import numpy as np

for _p in ("/opt/trn_rl_repo",):
    if os.path.isdir(_p) and _p not in sys.path:
        sys.path.insert(0, _p)

import concourse.bass as bass  # noqa: E402
import concourse.bass_isa as bass_isa  # noqa: E402
import concourse.mybir as mybir  # noqa: E402
import concourse.tile as tile  # noqa: E402
from concourse import bacc  # noqa: E402
from concourse.bass_utils import run_bass_kernel_spmd  # noqa: E402

F32 = mybir.dt.float32
F16 = mybir.dt.float16
AFT = mybir.ActivationFunctionType

N_CORES = 8

# Set by test harness to capture a profile on the next kernel() call.
TRACE = False
LAST_EXEC_NS = None
LAST_RESULTS = None

B, S, D = 2, 2048, 3072
QH, HD, SC = 3, 128, 512
CT = D // 128          # 24 contraction tiles
KT = S // 128          # 16 key tiles
NSC = S // SC          # 4 token chunks
XG = 6                 # x tiles per DMA group
SCALE = 1.0 / math.sqrt(HD)


def build_program():
    nc = bacc.Bacc("TRN2", target_bir_lowering=False, debug=False,
                   num_devices=N_CORES)

    # All dram layouts host-repacked so every DMA has long contiguous
    # per-partition lines (fat packets) and needs no on-the-fly rearrange.
    xT = nc.declare_dram_parameter("xT", [B, NSC, 128, CT, SC], F16,
                                   isOutput=False)
    cosT = nc.declare_dram_parameter("cosT", [HD, S], F16, isOutput=False)
    sinT = nc.declare_dram_parameter("sinT", [HD, S], F16, isOutput=False)
    onesc = nc.declare_dram_parameter("onesc", [128, 1], F16, isOutput=False)
    onesr = nc.declare_dram_parameter("onesr", [1, 128], F16, isOutput=False)
    wq = nc.declare_dram_parameter("wq", [128, CT, QH * HD], F16,
                                   isOutput=False)
    wk = nc.declare_dram_parameter("wk", [128, CT, HD], F16, isOutput=False)
    wv = nc.declare_dram_parameter("wv", [128, CT, HD], F16, isOutput=False)
    wo = nc.declare_dram_parameter("wo", [128, QH, D], F16, isOutput=False)
    # 8 masked (qc, kt-pair) blocks of exp(mask)^T, fp16 {0,1}
    em2 = nc.declare_dram_parameter("em2", [128, 2 * NSC, 2, SC], F16,
                                    isOutput=False)
    ident = nc.declare_dram_parameter("ident", [128, 128], F16, isOutput=False)
    yT = nc.declare_dram_parameter("yT", [B, D, S], F16, isOutput=True)

    xT_ap, yT_ap = xT.ap(), yT.ap()

    with tile.TileContext(nc) as tc:
        from contextlib import ExitStack
        with ExitStack() as top:
            const = top.enter_context(tc.tile_pool(name="const", bufs=1))
            stream = top.enter_context(tc.tile_pool(name="stream", bufs=1))

            wq_sb = const.tile([128, CT, QH * HD], F16, name="wq_sb")
            wk_sb = const.tile([128, CT, HD], F16, name="wk_sb")
            wv_sb = const.tile([128, CT, HD], F16, name="wv_sb")
            wo_sb = const.tile([128, QH, D], F16, name="wo_sb")
            cos_sb = const.tile([128, S], F16, name="cos_sb")
            sin_sb = const.tile([128, S], F16, name="sin_sb")
            em_sb = const.tile([128, 2 * NSC, 2, SC], F16, name="em_sb")
            ident_sb = const.tile([128, 128], F16, name="ident_sb")
            ones_col = const.tile([128, 1], F16, name="ones_col")
            ones_row = const.tile([1, 128], F16, name="ones_row")

            # x chunk group tiles, cached so the next batch's first chunk
            # can be prefetched during the previous batch's attention.
            xg_cache = {}

            def prefetch_xgroups(b, sc, with_weights=False):
                key = (b, sc)
                if key not in xg_cache:
                    # Cold start uses fine 3-ct groups so the first matmuls
                    # gate on ~0.8MB of DMA instead of several MB.
                    plan = [3] * 8 if with_weights else [XG] * (CT // XG)
                    xgs, ct0 = [], 0
                    for n in plan:
                        gs = slice(ct0, ct0 + n)
                        if with_weights:
                            # rate-match weight and x arrival per group
                            nc.sync.dma_start(wq_sb[:, gs, :], wq.ap()[:, gs])
                            nc.sync.dma_start(wk_sb[:, gs, :], wk.ap()[:, gs])
                            nc.sync.dma_start(wv_sb[:, gs, :], wv.ap()[:, gs])
                        xg = stream.tile([128, n, SC], F16, tag="xg",
                                         bufs=4, name="xg",
                                         padded_shape=[128, XG, SC])
                        nc.sync.dma_start(xg[:], xT_ap[b, sc, :, gs, :])
                        xgs.append((xg, ct0, n))
                        ct0 += n
                    xg_cache[key] = xgs

            def get_xgroups(b, sc):
                prefetch_xgroups(b, sc)
                groups = xg_cache.pop((b, sc))
                xr_map = {}
                for xg, ct0, n in groups:
                    for k in range(n):
                        xr_map[ct0 + k] = xg[:, k, :]
                return xr_map

            prefetch_xgroups(0, 0, with_weights=True)

            def late_preloads(stage):
                # Staged behind the x chunks that precede their first use so
                # they never delay the proj matmul stream.
                if stage == 0:      # RoPE needs these from chunk 0
                    nc.sync.dma_start(cos_sb[:], cosT.ap())
                    nc.sync.dma_start(sin_sb[:], sinT.ap())
                    nc.sync.dma_start(ident_sb[:], ident.ap())
                    nc.sync.dma_start(ones_col[:], onesc.ap())
                    nc.sync.dma_start(ones_row[:], onesr.ap())
                elif stage == 1:    # attention needs these
                    nc.sync.dma_start(em_sb[:], em2.ap())
                    nc.sync.dma_start(wo_sb[:], wo.ap())

            for b in range(B):
                with ExitStack() as bctx:
                    bpool = bctx.enter_context(
                        tc.tile_pool(name=f"b{b}_persist", bufs=1))
                    K_cks = [bpool.tile([128, SC], F16, name=f"K_sb{b}_{s_}")
                             for s_ in range(NSC)]
                    V_cks = [bpool.tile([128, SC // 128, 128], F16,
                                        name=f"V_sb{b}_{s_}")
                             for s_ in range(NSC)]
                    Q_cks = [[bpool.tile([128, SC], F16,
                                         name=f"Q_sb{b}_{i}_{s_}")
                              for s_ in range(NSC)] for i in range(QH)]

                    # ---------------- QKV projection + RoPE ----------------
                    with ExitStack() as pctx:
                        pps = pctx.enter_context(
                            tc.tile_pool(name=f"b{b}_qkv_ps", bufs=1,
                                         space="PSUM"))
                        sp = pctx.enter_context(
                            tc.tile_pool(name=f"b{b}_qkv_sb", bufs=1))

                        for sc in range(NSC):
                            cs = slice(sc * SC, (sc + 1) * SC)
                            xgs = get_xgroups(b, sc)
                            if b == 0 and sc == 0:
                                late_preloads(0)
                            elif b == 0 and sc == NSC - 1:
                                late_preloads(1)

                            accs = [pps.tile([128, SC], F32, tag="qkvacc",
                                             bufs=QH + 2, name=f"acc{j}")
                                    for j in range(QH + 2)]
                            for ct in range(CT):
                                xr = xgs[ct]
                                st, sp_ = (ct == 0), (ct == CT - 1)
                                for j in range(QH):
                                    nc.tensor.matmul(
                                        accs[j][:],
                                        wq_sb[:, ct, j * HD:(j + 1) * HD],
                                        xr, start=st, stop=sp_)
                                nc.tensor.matmul(accs[QH][:], wk_sb[:, ct, :],
                                                 xr, start=st, stop=sp_)
                                nc.tensor.matmul(accs[QH + 1][:],
                                                 wv_sb[:, ct, :],
                                                 xr, start=st, stop=sp_)

                            # V first (fp16): copy out of PSUM, PE-transpose
                            # to [s, d]. Emitted before the RoPE copies so
                            # the PE transposes aren't queued behind them
                            # on the scalar engine at the phase tail.
                            vstage = sp.tile([128, SC], F16, tag="vst", bufs=2,
                                             name="vstage")
                            nc.scalar.copy(vstage[:], accs[QH + 1][:])
                            for j in range(SC // 128):
                                v_ps = pps.tile([128, 128], F16, tag="vtr",
                                                bufs=2, name="v_ps")
                                nc.tensor.transpose(
                                    v_ps[:], vstage[:, j * 128:(j + 1) * 128],
                                    ident_sb[:])
                                nc.vector.tensor_copy(
                                    V_cks[sc][:, j, :], v_ps[:])

                            # RoPE on the QH q-heads and the k head (fp16).
                            rope_dsts = [q_ck[sc][:] for q_ck in Q_cks]
                            rope_dsts.append(K_cks[sc][:])
                            for j, dst in enumerate(rope_dsts):
                                t_ps = accs[j]
                                t_sb = sp.tile([128, SC], F16, tag="tsb",
                                               bufs=5, name="t_sb")
                                nc.scalar.copy(t_sb[:], t_ps[:])
                                rot_sb = sp.tile([128, SC], F16, tag="rot",
                                                 bufs=4, name="rot_sb")
                                # rotate-half via partition-shifted DMA;
                                # sign of the first half folded into sinT.
                                nc.sync.dma_start(rot_sb[0:64, :],
                                                  t_sb[64:128, :])
                                nc.sync.dma_start(rot_sb[64:128, :],
                                                  t_sb[0:64, :])
                                tmp1 = sp.tile([128, SC], F16, tag="tmp1",
                                               bufs=4, name="tmp1")
                                nc.vector.tensor_mul(tmp1[:], t_sb[:],
                                                     cos_sb[:, cs])
                                tmp2 = sp.tile([128, SC], F16, tag="tmp2",
                                               bufs=4, name="tmp2")
                                nc.vector.tensor_mul(tmp2[:], rot_sb[:],
                                                     sin_sb[:, cs])
                                nc.vector.tensor_add(dst, tmp1[:], tmp2[:])

                    # ------------- attention + out-projection -------------
                    if b + 1 < B:
                        prefetch_xgroups(b + 1, 0)  # next batch's x
                    with ExitStack() as actx:
                        aps = actx.enter_context(
                            tc.tile_pool(name=f"b{b}_attn_ps", bufs=1,
                                         space="PSUM"))
                        asb = actx.enter_context(
                            tc.tile_pool(name=f"b{b}_attn_sb", bufs=1))

                        # Pending out-projection work, drained into the next
                        # q-chunk's (ACT-bound) score/AV loop.
                        pending = []  # list of thunks, one per mt unit
                        # Slot-deferred emissions (head-end normalization
                        # chains) so PE steps never head-of-line block.
                        sched = []    # [slots_left, thunk]

                        def defer(n, fn):
                            sched.append([n, fn])

                        def tick():
                            for item in sched[:]:
                                item[0] -= 1
                                if item[0] <= 0:
                                    sched.remove(item)
                                    item[1]()

                        # During final drains (k-loop finished) the s2/av
                        # PSUM banks are idle: borrow them so the out-proj
                        # pipeline isn't throttled by 2 y-banks against the
                        # ~620ns PSUM->SBUF copies.
                        deep_drain = [False]

                        def emit_oproj(qc, ohs):
                            qs = slice(qc * SC, (qc + 1) * SC)

                            def unit(mt, qs=qs, ohs=ohs):
                                if deep_drain[0] and mt % 3 == 1:
                                    y_ps = aps.tile([128, SC], F32, tag="av",
                                                    bufs=2, name="y_ps")
                                elif deep_drain[0] and mt % 3 == 2:
                                    y2 = aps.tile([128, 2, SC], F32, tag="s2",
                                                  bufs=2, name="y2_ps")
                                    y_ps = y2[:, 0, :]
                                else:
                                    y_ps = aps.tile([128, SC], F32, tag="y",
                                                    bufs=2, name="y_ps")
                                for hh in range(QH):
                                    nc.tensor.matmul(
                                        y_ps[:],
                                        wo_sb[:, hh, mt * 128:(mt + 1) * 128],
                                        ohs[hh][:],
                                        start=(hh == 0), stop=(hh == QH - 1))
                                y_sb = asb.tile([128, SC], F16, tag="yout",
                                                bufs=8, name="y_sb")
                                if mt % 2 == 0:
                                    nc.vector.tensor_copy(y_sb[:], y_ps[:])
                                else:
                                    nc.scalar.copy(y_sb[:], y_ps[:])
                                nc.sync.dma_start(
                                    yT_ap[b, mt * 128:(mt + 1) * 128, qs],
                                    y_sb[:])
                            for mt in range(CT):
                                pending.append(lambda mt=mt: unit(mt))

                        def drain(n):
                            for _ in range(min(n, len(pending))):
                                pending.pop(0)()

                        for qc in range(NSC):
                            npair = 2 * qc + 2   # kt pairs; last 2 masked
                            # pair-slots in this qc (3 heads); hold off
                            # draining for the first few so the previous
                            # q-chunk's oh normalization latency is hidden.
                            slots = 3 * npair
                            hold = 5
                            for hh in range(QH):
                                av_ps = aps.tile([128, SC], F32, tag="av",
                                                 bufs=2, name="av_ps")
                                E2_acc = asb.tile([128, 2, SC], F16,
                                                  tag="eacc", bufs=2,
                                                  name="E2_acc")
                                for pi in range(npair):
                                    kt0 = 2 * pi
                                    masked = pi >= npair - 2
                                    s2 = aps.tile([128, 2, SC], F32, tag="s2",
                                                  bufs=2, name="s2")
                                    for j in range(2):
                                        kb, kj = divmod(kt0 + j, SC // 128)
                                        nc.tensor.matmul(
                                            s2[:, j, :],
                                            K_cks[kb][:, kj * 128:
                                                      (kj + 1) * 128],
                                            Q_cks[hh][qc][:],
                                            start=True, stop=True)
                                    if pi == 0:
                                        e2 = E2_acc  # exp lands in the accum
                                    else:
                                        e2 = asb.tile([128, 2, SC], F16,
                                                      tag="e2", bufs=4,
                                                      name="e2")
                                    if masked:
                                        e_st = asb.tile([128, 2, SC], F16,
                                                        tag="est", bufs=2,
                                                        name="e_st")
                                        nc.scalar.activation(
                                            e_st[:], s2[:], AFT.Exp,
                                            scale=SCALE)
                                        mp = 2 * qc + (pi - (npair - 2))
                                        nc.vector.tensor_mul(
                                            e2[:], e_st[:],
                                            em_sb[:, mp, :, :])
                                    else:
                                        nc.scalar.activation(
                                            e2[:], s2[:], AFT.Exp,
                                            scale=SCALE)
                                    for j in range(2):
                                        kb, kj = divmod(kt0 + j, SC // 128)
                                        nc.tensor.matmul(
                                            av_ps[:], V_cks[kb][:, kj, :],
                                            e2[:, j, :],
                                            start=(pi == 0 and j == 0),
                                            stop=(pi == npair - 1 and j == 1))
                                    if pi > 0:
                                        nc.vector.tensor_add(
                                            E2_acc[:], E2_acc[:], e2[:])
                                    # overlap pending out-proj with this
                                    # ACT-bound loop
                                    tick()
                                    if pending and hold <= 0:
                                        drain(-(-len(pending) // slots))
                                    slots -= 1
                                    hold -= 1

                                # Head-end softmax normalization: rowsum via
                                # ones-matmuls (PSUM banks borrowed from the
                                # y tag), reciprocal, row-broadcast matmul,
                                # normalize. Each PE step is deferred by
                                # pair-slots so it lands in the stream only
                                # after its inputs are surely ready.
                                oh = asb.tile([128, SC], F16, tag="oh",
                                              bufs=2 * QH + 1, name="oh")
                                if hh == 0:
                                    ohs = []
                                ohs.append(oh)

                                def chain(E2_acc=E2_acc, av_ps=av_ps, oh=oh):
                                    r_t = aps.tile([128, SC], F32, tag="y",
                                                   bufs=2, name="r_ps")
                                    invf = asb.tile([1, SC], F32, tag="invf",
                                                    bufs=2, name="invf")
                                    inv = asb.tile([1, SC], F16, tag="inv",
                                                   bufs=2, name="inv")
                                    invb = asb.tile([128, SC], F32, tag="rb",
                                                    bufs=2, name="invb")

                                    def st_r():
                                        nc.tensor.matmul(
                                            r_t[0:1, :], ones_col[:],
                                            E2_acc[:, 0, :],
                                            start=True, stop=False)
                                        nc.tensor.matmul(
                                            r_t[0:1, :], ones_col[:],
                                            E2_acc[:, 1, :],
                                            start=False, stop=True)

                                    def st_recip():
                                        nc.vector.reciprocal_approx_fast(
                                            invf[:], r_t[0:1, :])
                                        nc.vector.tensor_copy(inv[:], invf[:])

                                    def st_invb():
                                        invb_t = aps.tile(
                                            [128, SC], F32, tag="y", bufs=2,
                                            name="invb_ps")
                                        nc.tensor.matmul(
                                            invb_t[:], ones_row[:], inv[:],
                                            start=True, stop=True)
                                        nc.scalar.copy(invb[:], invb_t[:])

                                    def st_oh():
                                        nc.vector.tensor_mul(
                                            oh[:], av_ps[:], invb[:])

                                    return [st_r, st_recip, st_invb, st_oh]

                                for i, st in enumerate(chain()):
                                    defer(i + 1, st)

                            emit_oproj(qc, ohs)

                        while sched:
                            tick()
                        deep_drain[0] = True
                        drain(len(pending))  # last q-chunk's out-proj

    nc.compile()
    return nc


def make_inputs(x, freqs_cos, freqs_sin, mask, wq, wk, wv, wo):
    """Host-side preprocessing -> per-core input maps (repacked layouts)."""
    f32, f16 = np.float32, np.float16
    x = np.asarray(x, f32)
    xT = np.transpose(x, (0, 2, 1)).astype(f16)          # [B, D, S]
    # -> [B, NSC, 128, CT, SC]: fat contiguous per-partition DMA lines
    xTr = np.ascontiguousarray(
        xT.reshape(B, CT, 128, NSC, SC).transpose(0, 3, 2, 1, 4))
    cosT = np.ascontiguousarray(
        np.concatenate([freqs_cos, freqs_cos], axis=1).T.astype(f16))
    sinT = np.concatenate([freqs_sin, freqs_sin], axis=1).T.astype(f32).copy()
    sinT[:HD // 2] *= -1.0  # sign of rotate-half folded in
    sinT = np.ascontiguousarray(sinT.astype(f16))

    em = np.exp(np.asarray(mask, f32)[0, 0]).T  # [k, q] multiplicative
    em2 = np.zeros((2 * NSC, 128, 2, SC), f16)
    for qc in range(NSC):
        for p in range(2):
            for j in range(2):
                kt = 4 * qc + 2 * p + j
                em2[2 * qc + p, :, j, :] = em[
                    kt * 128:(kt + 1) * 128, qc * SC:(qc + 1) * SC]
    em2r = np.ascontiguousarray(em2.transpose(1, 0, 2, 3))
    identity = np.ascontiguousarray(np.eye(128, dtype=f16))

    wqT = np.asarray(wq, f32).T.astype(f16)
    wkT = np.asarray(wk, f32).T.astype(f16)
    wvT = np.asarray(wv, f32).T.astype(f16)
    woT = np.asarray(wo, f32).T.astype(f16)

    def pack_w(wt):  # [D, m] -> [128, CT, m]
        return np.ascontiguousarray(
            wt.reshape(-1, 128, wt.shape[1]).transpose(1, 0, 2))

    in_maps = []
    for h in range(N_CORES):
        qsl = slice(h * QH * HD, (h + 1) * QH * HD)
        ksl = slice(h * HD, (h + 1) * HD)
        in_maps.append({
            "xT": xTr,
            "cosT": cosT,
            "sinT": sinT,
            "wq": pack_w(wqT[:, qsl]),
            "wk": pack_w(wkT[:, ksl]),
            "wv": pack_w(wvT[:, ksl]),
            "wo": pack_w(woT[qsl, :]),   # [384, D] -> [128, QH, D]
            "em2": em2r,
            "ident": identity,
            "onesc": np.ones((128, 1), f16),
            "onesr": np.ones((1, 128), f16),
        })
    return in_maps


_CACHE = {}


def kernel(x, freqs_cos, freqs_sin, mask, wq, wk, wv, wo):
    global LAST_EXEC_NS, LAST_RESULTS
    assert tuple(x.shape) == (B, S, D), x.shape

    in_maps = make_inputs(x, freqs_cos, freqs_sin, mask, wq, wk, wv, wo)

    if "prog" not in _CACHE:
        _CACHE["prog"] = build_program()
    nc = _CACHE["prog"]

    kwargs = {}
    if TRACE:
        kwargs = dict(trace=True, trace_cores=[0])
    res = run_bass_kernel_spmd(nc, in_maps, list(range(N_CORES)), **kwargs)
    LAST_EXEC_NS = res.exec_time_ns
    LAST_RESULTS = res

    acc = np.zeros((B, D, S), np.float32)
    for i in range(N_CORES):
        acc += res.results[i]["yT"].astype(np.float32)
    y = np.ascontiguousarray(np.transpose(acc, (0, 2, 1)).astype(np.float32))
    return y


# revision 46
# speedup vs baseline: 1.2365x; 1.0178x over previous
"""GQA attention kernel for Trainium2 (8 NeuronCores, Bass/Tile).

Problem: B=2, S=2048, D=3072, 24 Q heads / 8 KV heads, HD=128, RoPE,
additive causal mask, softmax, output projection.

Sharding: tensor-parallel over heads. Core h owns KV head h and Q heads
{3h, 3h+1, 3h+2} for BOTH batch elements. Each core produces a partial
y^T = wo_slice^T.T @ attn_out_heads^T of shape (B, D, S) in fp16; the
host sums the 8 partials in fp32 and transposes back.

Layout: everything transposed ([feature, token]) on chip so every
matmul contracts on the partition dim with a 512-wide fp16 moving
operand (1 cycle/row on the PE):
  - x^T streamed from DRAM (host pre-transposed, fp16)
  - QKV projection -> Q^T,K^T [hd, S] per head; RoPE in transposed
    layout (rotate-half via partition-shifted SBUF DMA, sign folded
    into sinT)
  - scores^T [k, q] = K-tile @ Q^T into PAIRED PSUM banks; ONE exp per
    pair on ACT ([128,2,512]) with the 1/sqrt(HD) scale folded in;
    causal mask applied multiplicatively (fp16, DVE 4x mode)
  - attn@V accumulated in PSUM; softmax denominator: DVE accumulates
    exp pairs (fp16), then a slot-DEFERRED ones-matmul rowsum +
    reciprocal + ones-row broadcast matmul normalize without ever
    head-of-line blocking the PE
  - out-projection matmuls interleaved into the NEXT q-chunk's
    score/AV loop so the ACT-bound k-loop and PE-bound out-proj
    overlap; host-repacked dram layouts give every DMA fat contiguous
    lines, with first-chunk weight/x groups interleaved for cold start
"""

import math
import os
import sys

import numpy as np

for _p in ("/opt/trn_rl_repo",):
    if os.path.isdir(_p) and _p not in sys.path:
        sys.path.insert(0, _p)

import concourse.bass as bass  # noqa: E402
import concourse.bass_isa as bass_isa  # noqa: E402
import concourse.mybir as mybir  # noqa: E402
import concourse.tile as tile  # noqa: E402
from concourse import bacc  # noqa: E402
from concourse.bass_utils import run_bass_kernel_spmd  # noqa: E402

F32 = mybir.dt.float32
F16 = mybir.dt.float16
AFT = mybir.ActivationFunctionType

N_CORES = 8

# Set by test harness to capture a profile on the next kernel() call.
TRACE = False
LAST_EXEC_NS = None
LAST_RESULTS = None

B, S, D = 2, 2048, 3072
QH, HD, SC = 3, 128, 512
CT = D // 128          # 24 contraction tiles
KT = S // 128          # 16 key tiles
NSC = S // SC          # 4 token chunks
XG = 6                 # x tiles per DMA group
SCALE = 1.0 / math.sqrt(HD)


def build_program():
    nc = bacc.Bacc("TRN2", target_bir_lowering=False, debug=False,
                   num_devices=N_CORES)

    # All dram layouts host-repacked so every DMA has long contiguous
    # per-partition lines (fat packets) and needs no on-the-fly rearrange.
    xT = nc.declare_dram_parameter("xT", [B, NSC, 128, CT, SC], F16,
                                   isOutput=False)
    cosT = nc.declare_dram_parameter("cosT", [HD, S], F16, isOutput=False)
    sinT = nc.declare_dram_parameter("sinT", [HD, S], F16, isOutput=False)
    onesc = nc.declare_dram_parameter("onesc", [128, 1], F16, isOutput=False)
    onesr = nc.declare_dram_parameter("onesr", [1, 128], F16, isOutput=False)
    wq = nc.declare_dram_parameter("wq", [128, CT, QH * HD], F16,
                                   isOutput=False)
    wk = nc.declare_dram_parameter("wk", [128, CT, HD], F16, isOutput=False)
    wv = nc.declare_dram_parameter("wv", [128, CT, HD], F16, isOutput=False)
    wo = nc.declare_dram_parameter("wo", [128, QH, D], F16, isOutput=False)
    # 8 masked (qc, kt-pair) blocks of exp(mask)^T, fp16 {0,1}
    em2 = nc.declare_dram_parameter("em2", [128, 2 * NSC, 2, SC], F16,
                                    isOutput=False)
    ident = nc.declare_dram_parameter("ident", [128, 128], F16, isOutput=False)
    yT = nc.declare_dram_parameter("yT", [B, D, S], F16, isOutput=True)

    xT_ap, yT_ap = xT.ap(), yT.ap()

    with tile.TileContext(nc) as tc:
        from contextlib import ExitStack
        with ExitStack() as top:
            const = top.enter_context(tc.tile_pool(name="const", bufs=1))
            stream = top.enter_context(tc.tile_pool(name="stream", bufs=1))

            wq_sb = const.tile([128, CT, QH * HD], F16, name="wq_sb")
            wk_sb = const.tile([128, CT, HD], F16, name="wk_sb")
            wv_sb = const.tile([128, CT, HD], F16, name="wv_sb")
            wo_sb = const.tile([128, QH, D], F16, name="wo_sb")
            cos_sb = const.tile([128, S], F16, name="cos_sb")
            sin_sb = const.tile([128, S], F16, name="sin_sb")
            em_sb = const.tile([128, 2 * NSC, 2, SC], F16, name="em_sb")
            ident_sb = const.tile([128, 128], F16, name="ident_sb")
            ones_col = const.tile([128, 1], F16, name="ones_col")
            ones_row = const.tile([1, 128], F16, name="ones_row")

            # x chunk group tiles, cached so the next batch's first chunk
            # can be prefetched during the previous batch's attention.
            xg_cache = {}

            def prefetch_xgroups(b, sc, with_weights=False):
                key = (b, sc)
                if key not in xg_cache:
                    # Cold start uses fine 3-ct groups so the first matmuls
                    # gate on ~0.8MB of DMA instead of several MB.
                    plan = [3] * 8 if with_weights else [XG] * (CT // XG)
                    xgs, ct0 = [], 0
                    for n in plan:
                        gs = slice(ct0, ct0 + n)
                        if with_weights:
                            # rate-match weight and x arrival per group
                            nc.sync.dma_start(wq_sb[:, gs, :], wq.ap()[:, gs])
                            nc.sync.dma_start(wk_sb[:, gs, :], wk.ap()[:, gs])
                            nc.sync.dma_start(wv_sb[:, gs, :], wv.ap()[:, gs])
                        xg = stream.tile([128, n, SC], F16, tag="xg",
                                         bufs=4, name="xg",
                                         padded_shape=[128, XG, SC])
                        nc.sync.dma_start(xg[:], xT_ap[b, sc, :, gs, :])
                        xgs.append((xg, ct0, n))
                        ct0 += n
                    xg_cache[key] = xgs

            def get_xgroups(b, sc):
                prefetch_xgroups(b, sc)
                groups = xg_cache.pop((b, sc))
                xr_map = {}
                for xg, ct0, n in groups:
                    for k in range(n):
                        xr_map[ct0 + k] = xg[:, k, :]
                return xr_map

            prefetch_xgroups(0, 0, with_weights=True)

            def late_preloads(stage):
                # Staged behind the x chunks that precede their first use so
                # they never delay the proj matmul stream.
                if stage == 0:      # RoPE needs these from chunk 0
                    nc.sync.dma_start(cos_sb[:], cosT.ap())
                    nc.sync.dma_start(sin_sb[:], sinT.ap())
                    nc.sync.dma_start(ident_sb[:], ident.ap())
                    nc.sync.dma_start(ones_col[:], onesc.ap())
                    nc.sync.dma_start(ones_row[:], onesr.ap())
                elif stage == 1:    # attention needs these
                    nc.sync.dma_start(em_sb[:], em2.ap())
                    nc.sync.dma_start(wo_sb[:], wo.ap())

            # One kernel-wide PSUM/SBUF pool pair; the proj phase borrows
            # the attention tags (s2/av for accumulators, y for the V
            # transpose) so out-proj units held back from batch 0 can
            # drain into batch 1's ACT-bound first q-chunk.
            aps = top.enter_context(
                tc.tile_pool(name="attn_ps", bufs=1, space="PSUM"))
            asb = top.enter_context(tc.tile_pool(name="attn_sb", bufs=1))
            sp = top.enter_context(tc.tile_pool(name="proj_sb", bufs=1))

            # Pending out-projection specs (b, qs, ohs, mt), drained into
            # the next q-chunk's (ACT-bound) score/AV loop.
            pending = []
            # Slot-deferred emissions (head-end normalization chains) so
            # PE steps never head-of-line block.
            sched = []    # [slots_left, thunk]

            def defer(n, fn):
                sched.append([n, fn])

            def tick():
                for item in sched[:]:
                    item[0] -= 1
                    if item[0] <= 0:
                        sched.remove(item)
                        item[1]()

            # During tail drains (k-loop finished) the s2/av PSUM banks
            # are idle: borrow them so the out-proj pipeline isn't
            # throttled by 2 y-banks against the ~620ns PSUM->SBUF copies.
            deep_drain = [False]
            unit_k = [0]

            def alloc_yps():
                k = unit_k[0]
                unit_k[0] += 1
                if deep_drain[0] and k % 3 == 1:
                    return aps.tile([128, SC], F32, tag="av", bufs=2,
                                    name="y_ps")
                if deep_drain[0] and k % 3 == 2:
                    y2 = aps.tile([128, 2, SC], F32, tag="s2", bufs=2,
                                  name="y2_ps")
                    return y2[:, 0, :]
                return aps.tile([128, SC], F32, tag="y", bufs=2, name="y_ps")

            def unit_mm(spec, y_ps, hh, start, stop):
                _b, _qs, _ohs, mt = spec
                nc.tensor.matmul(
                    y_ps[:], wo_sb[:, hh, mt * 128:(mt + 1) * 128],
                    _ohs[hh][:], start=start, stop=stop)

            def unit_out(spec, y_ps):
                _b, _qs, _ohs, mt = spec
                y_sb = asb.tile([128, SC], F16, tag="yout", bufs=8,
                                name="y_sb")
                if mt % 2 == 0:
                    nc.vector.tensor_copy(y_sb[:], y_ps[:])
                else:
                    nc.scalar.copy(y_sb[:], y_ps[:])
                nc.sync.dma_start(
                    yT_ap[_b, mt * 128:(mt + 1) * 128, _qs], y_sb[:])

            def emit_unit(spec):
                y_ps = alloc_yps()
                for hh in range(QH):
                    unit_mm(spec, y_ps, hh, hh == 0, hh == QH - 1)
                unit_out(spec, y_ps)

            def drain(n):
                for _ in range(min(n, len(pending))):
                    emit_unit(pending.pop(0))

            def tail_drain(keep=0):
                # Two-pass head start: the first units' h0/h1 matmuls only
                # need the EARLY heads' oh, so they execute while the last
                # head's normalization chain is still in flight.
                deep_drain[0] = True
                head = [pending.pop(0)
                        for _ in range(min(6, max(0, len(pending) - keep)))]
                held = [(spec, alloc_yps()) for spec in head]
                for spec, y_ps in held:
                    unit_mm(spec, y_ps, 0, True, False)
                    unit_mm(spec, y_ps, 1, False, False)
                for spec, y_ps in held:
                    unit_mm(spec, y_ps, 2, False, True)
                    unit_out(spec, y_ps)
                drain(len(pending) - keep)
                deep_drain[0] = False

            for b in range(B):
                with ExitStack() as bctx:
                    bpool = bctx.enter_context(
                        tc.tile_pool(name=f"b{b}_persist", bufs=1))
                    K_cks = [bpool.tile([128, SC], F16, name=f"K_sb{b}_{s_}")
                             for s_ in range(NSC)]
                    V_cks = [bpool.tile([128, SC // 128, 128], F16,
                                        name=f"V_sb{b}_{s_}")
                             for s_ in range(NSC)]
                    Q_cks = [[bpool.tile([128, SC], F16,
                                         name=f"Q_sb{b}_{i}_{s_}")
                              for s_ in range(NSC)] for i in range(QH)]

                    # ---------------- QKV projection + RoPE ----------------
                    if True:
                        for sc in range(NSC):
                            cs = slice(sc * SC, (sc + 1) * SC)
                            xgs = get_xgroups(b, sc)
                            if b == 0 and sc == 0:
                                late_preloads(0)
                            elif b == 0 and sc == NSC - 1:
                                late_preloads(1)

                            accA = aps.tile([128, 2, SC], F32, tag="s2",
                                            bufs=2, name="accA")
                            accB = aps.tile([128, 2, SC], F32, tag="s2",
                                            bufs=2, name="accB")
                            accC = aps.tile([128, SC], F32, tag="av",
                                            bufs=2, name="accC")
                            accs = [accA[:, 0, :], accA[:, 1, :],
                                    accB[:, 0, :], accB[:, 1, :], accC[:]]
                            for ct in range(CT):
                                xr = xgs[ct]
                                st, sp_ = (ct == 0), (ct == CT - 1)
                                for j in range(QH):
                                    nc.tensor.matmul(
                                        accs[j][:],
                                        wq_sb[:, ct, j * HD:(j + 1) * HD],
                                        xr, start=st, stop=sp_)
                                nc.tensor.matmul(accs[QH][:], wk_sb[:, ct, :],
                                                 xr, start=st, stop=sp_)
                                nc.tensor.matmul(accs[QH + 1][:],
                                                 wv_sb[:, ct, :],
                                                 xr, start=st, stop=sp_)

                            # V first (fp16): copy out of PSUM, PE-transpose
                            # to [s, d]. Emitted before the RoPE copies so
                            # the PE transposes aren't queued behind them
                            # on the scalar engine at the phase tail.
                            vstage = sp.tile([128, SC], F16, tag="vst", bufs=2,
                                             name="vstage")
                            nc.scalar.copy(vstage[:], accs[QH + 1][:])
                            for j in range(SC // 128):
                                v_t = aps.tile([128, SC], F32, tag="y",
                                               bufs=2, name="v_t")
                                v_ps = v_t.bitcast(F16)[:, 0:128]
                                nc.tensor.transpose(
                                    v_ps, vstage[:, j * 128:(j + 1) * 128],
                                    ident_sb[:])
                                nc.vector.tensor_copy(
                                    V_cks[sc][:, j, :], v_ps)

                            # RoPE on the QH q-heads and the k head (fp16).
                            rope_dsts = [q_ck[sc][:] for q_ck in Q_cks]
                            rope_dsts.append(K_cks[sc][:])
                            for j, dst in enumerate(rope_dsts):
                                t_ps = accs[j]
                                t_sb = sp.tile([128, SC], F16, tag="tsb",
                                               bufs=5, name="t_sb")
                                nc.scalar.copy(t_sb[:], t_ps[:])
                                rot_sb = sp.tile([128, SC], F16, tag="rot",
                                                 bufs=4, name="rot_sb")
                                # rotate-half via partition-shifted DMA;
                                # sign of the first half folded into sinT.
                                nc.sync.dma_start(rot_sb[0:64, :],
                                                  t_sb[64:128, :])
                                nc.sync.dma_start(rot_sb[64:128, :],
                                                  t_sb[0:64, :])
                                tmp1 = sp.tile([128, SC], F16, tag="tmp1",
                                               bufs=4, name="tmp1")
                                nc.vector.tensor_mul(tmp1[:], t_sb[:],
                                                     cos_sb[:, cs])
                                tmp2 = sp.tile([128, SC], F16, tag="tmp2",
                                               bufs=4, name="tmp2")
                                nc.vector.tensor_mul(tmp2[:], rot_sb[:],
                                                     sin_sb[:, cs])
                                nc.vector.tensor_add(dst, tmp1[:], tmp2[:])

                    # ------------- attention + out-projection -------------
                    if b + 1 < B:
                        prefetch_xgroups(b + 1, 0)  # next batch's x
                    if True:
                        for qc in range(NSC):
                            npair = 2 * qc + 2   # kt pairs; last 2 masked
                            # pair-slots in this qc (3 heads); hold off
                            # draining for the first few so the previous
                            # q-chunk's oh normalization latency is hidden.
                            # (Units held over from the previous batch are
                            # long since ready -> no hold.)
                            slots = 3 * npair
                            hold = 0 if qc == 0 else 5
                            for hh in range(QH):
                                av_ps = aps.tile([128, SC], F32, tag="av",
                                                 bufs=2, name="av_ps")
                                E2_acc = asb.tile([128, 2, SC], F16,
                                                  tag="eacc", bufs=2,
                                                  name="E2_acc")
                                for pi in range(npair):
                                    kt0 = 2 * pi
                                    masked = pi >= npair - 2
                                    s2 = aps.tile([128, 2, SC], F32, tag="s2",
                                                  bufs=2, name="s2")
                                    for j in range(2):
                                        kb, kj = divmod(kt0 + j, SC // 128)
                                        nc.tensor.matmul(
                                            s2[:, j, :],
                                            K_cks[kb][:, kj * 128:
                                                      (kj + 1) * 128],
                                            Q_cks[hh][qc][:],
                                            start=True, stop=True)
                                    if pi == 0:
                                        e2 = E2_acc  # exp lands in the accum
                                    else:
                                        e2 = asb.tile([128, 2, SC], F16,
                                                      tag="e2", bufs=4,
                                                      name="e2")
                                    if masked:
                                        e_st = asb.tile([128, 2, SC], F16,
                                                        tag="est", bufs=2,
                                                        name="e_st")
                                        nc.scalar.activation(
                                            e_st[:], s2[:], AFT.Exp,
                                            scale=SCALE)
                                        mp = 2 * qc + (pi - (npair - 2))
                                        nc.vector.tensor_mul(
                                            e2[:], e_st[:],
                                            em_sb[:, mp, :, :])
                                    else:
                                        nc.scalar.activation(
                                            e2[:], s2[:], AFT.Exp,
                                            scale=SCALE)
                                    for j in range(2):
                                        kb, kj = divmod(kt0 + j, SC // 128)
                                        nc.tensor.matmul(
                                            av_ps[:], V_cks[kb][:, kj, :],
                                            e2[:, j, :],
                                            start=(pi == 0 and j == 0),
                                            stop=(pi == npair - 1 and j == 1))
                                    if pi > 0:
                                        nc.vector.tensor_add(
                                            E2_acc[:], E2_acc[:], e2[:])
                                    # overlap pending out-proj with this
                                    # ACT-bound loop
                                    tick()
                                    if pending and hold <= 0:
                                        drain(-(-len(pending) // slots))
                                    slots -= 1
                                    hold -= 1

                                # Head-end softmax normalization: rowsum via
                                # ones-matmuls (PSUM banks borrowed from the
                                # y tag), reciprocal, row-broadcast matmul,
                                # normalize. Each PE step is deferred by
                                # pair-slots so it lands in the stream only
                                # after its inputs are surely ready.
                                oh = asb.tile([128, SC], F16, tag="oh",
                                              bufs=2 * QH + 1, name="oh")
                                if hh == 0:
                                    ohs = []
                                ohs.append(oh)

                                def chain(E2_acc=E2_acc, av_ps=av_ps, oh=oh):
                                    r_t = aps.tile([128, SC], F32, tag="y",
                                                   bufs=2, name="r_ps")
                                    invf = asb.tile([1, SC], F32, tag="invf",
                                                    bufs=2, name="invf")
                                    inv = asb.tile([1, SC], F16, tag="inv",
                                                   bufs=2, name="inv")
                                    invb = asb.tile([128, SC], F32, tag="rb",
                                                    bufs=2, name="invb")

                                    def st_r():
                                        nc.tensor.matmul(
                                            r_t[0:1, :], ones_col[:],
                                            E2_acc[:, 0, :],
                                            start=True, stop=False)
                                        nc.tensor.matmul(
                                            r_t[0:1, :], ones_col[:],
                                            E2_acc[:, 1, :],
                                            start=False, stop=True)

                                    def st_recip():
                                        nc.vector.reciprocal_approx_fast(
                                            invf[:], r_t[0:1, :])
                                        nc.vector.tensor_copy(inv[:], invf[:])

                                    def st_invb():
                                        invb_t = aps.tile(
                                            [128, SC], F32, tag="y", bufs=2,
                                            name="invb_ps")
                                        nc.tensor.matmul(
                                            invb_t[:], ones_row[:], inv[:],
                                            start=True, stop=True)
                                        nc.scalar.copy(invb[:], invb_t[:])

                                    def st_oh():
                                        nc.vector.tensor_mul(
                                            oh[:], av_ps[:], invb[:])

                                    return [st_r, st_recip, st_invb, st_oh]

                                for i, st in enumerate(chain()):
                                    defer(i + 1, st)

                            qs = slice(qc * SC, (qc + 1) * SC)
                            for mt in range(CT):
                                pending.append((b, qs, ohs, mt))

                        while sched:
                            tick()
                        # Keep 8 units of the last q-chunk to fill the next
                        # batch's ACT-bound first q-chunk (PE filler there).
                        tail_drain(keep=8 if b + 1 < B else 0)

    nc.compile()
    return nc


def make_inputs(x, freqs_cos, freqs_sin, mask, wq, wk, wv, wo):
    """Host-side preprocessing -> per-core input maps (repacked layouts)."""
    f32, f16 = np.float32, np.float16
    x = np.asarray(x, f32)
    xT = np.transpose(x, (0, 2, 1)).astype(f16)          # [B, D, S]
    # -> [B, NSC, 128, CT, SC]: fat contiguous per-partition DMA lines
    xTr = np.ascontiguousarray(
        xT.reshape(B, CT, 128, NSC, SC).transpose(0, 3, 2, 1, 4))
    cosT = np.ascontiguousarray(
        np.concatenate([freqs_cos, freqs_cos], axis=1).T.astype(f16))
    sinT = np.concatenate([freqs_sin, freqs_sin], axis=1).T.astype(f32).copy()
    sinT[:HD // 2] *= -1.0  # sign of rotate-half folded in
    sinT = np.ascontiguousarray(sinT.astype(f16))

    em = np.exp(np.asarray(mask, f32)[0, 0]).T  # [k, q] multiplicative
    em2 = np.zeros((2 * NSC, 128, 2, SC), f16)
    for qc in range(NSC):
        for p in range(2):
            for j in range(2):
                kt = 4 * qc + 2 * p + j
                em2[2 * qc + p, :, j, :] = em[
                    kt * 128:(kt + 1) * 128, qc * SC:(qc + 1) * SC]
    em2r = np.ascontiguousarray(em2.transpose(1, 0, 2, 3))
    identity = np.ascontiguousarray(np.eye(128, dtype=f16))

    wqT = np.asarray(wq, f32).T.astype(f16)
    wkT = np.asarray(wk, f32).T.astype(f16)
    wvT = np.asarray(wv, f32).T.astype(f16)
    woT = np.asarray(wo, f32).T.astype(f16)

    def pack_w(wt):  # [D, m] -> [128, CT, m]
        return np.ascontiguousarray(
            wt.reshape(-1, 128, wt.shape[1]).transpose(1, 0, 2))

    in_maps = []
    for h in range(N_CORES):
        qsl = slice(h * QH * HD, (h + 1) * QH * HD)
        ksl = slice(h * HD, (h + 1) * HD)
        in_maps.append({
            "xT": xTr,
            "cosT": cosT,
            "sinT": sinT,
            "wq": pack_w(wqT[:, qsl]),
            "wk": pack_w(wkT[:, ksl]),
            "wv": pack_w(wvT[:, ksl]),
            "wo": pack_w(woT[qsl, :]),   # [384, D] -> [128, QH, D]
            "em2": em2r,
            "ident": identity,
            "onesc": np.ones((128, 1), f16),
            "onesr": np.ones((1, 128), f16),
        })
    return in_maps


_CACHE = {}


def kernel(x, freqs_cos, freqs_sin, mask, wq, wk, wv, wo):
    global LAST_EXEC_NS, LAST_RESULTS
    assert tuple(x.shape) == (B, S, D), x.shape

    in_maps = make_inputs(x, freqs_cos, freqs_sin, mask, wq, wk, wv, wo)

    if "prog" not in _CACHE:
        _CACHE["prog"] = build_program()
    nc = _CACHE["prog"]

    kwargs = {}
    if TRACE:
        kwargs = dict(trace=True, trace_cores=[0])
    res = run_bass_kernel_spmd(nc, in_maps, list(range(N_CORES)), **kwargs)
    LAST_EXEC_NS = res.exec_time_ns
    LAST_RESULTS = res

    acc = np.zeros((B, D, S), np.float32)
    for i in range(N_CORES):
        acc += res.results[i]["yT"].astype(np.float32)
    y = np.ascontiguousarray(np.transpose(acc, (0, 2, 1)).astype(np.float32))
    return y
